# revision 6
# baseline (speedup 1.0000x reference)
# Trainium2 Bass kernel for nn_Attention_81028853007030
#
# Model: 1-unit LSTM over [B=64, L=2048, E=300] -> scores -> (buggy) mask ->
# softmax over L -> attn * x.
#
# Strategy:
#   - Pure data parallel over 8 cores (8 sequences per core).
#   - The LSTM recurrence (H=1) is chunked in time: each (seq, chunk-of-128)
#     pair becomes one of 128 SBUF partitions; every chunk re-runs a 32-step
#     warmup from zero state. Forget-gate decay makes the truncated state
#     influence < fp32 rounding (validated offline: h abs err ~7e-6, same as a
#     full fp32 scan).
#   - xg = x @ W_ih^T + b is computed on the TensorEngine (PE transpose of x
#     tiles + matmul), pipelined with the serial scan.
#   - Softmax per sequence with fused exp+sum on the scalar engine.
#   - Final attn*x multiply runs in-place on the resident x tile.

import numpy as np

B, L, E = 64, 2048, 300
NCORES = 8
S = B // NCORES          # sequences per core
T = 128                  # chunk length
NCH = L // T             # chunks per sequence
V = S * NCH              # virtual sequences per core = 128 partitions
W = 32                   # warmup steps
U = W + T                # scan steps
ECH = [(0, 128), (128, 128), (256, 44)]  # E-chunks for the matmul
NEG = -1.0e30

_CACHE = {}


def _build_nc():
    from contextlib import ExitStack

    import concourse.bacc as bacc
    import concourse.mybir as mybir
    from concourse import tile
    from concourse.masks import make_identity

    F32 = mybir.dt.float32
    I32 = mybir.dt.int32
    Alu = mybir.AluOpType
    Act = mybir.ActivationFunctionType

    nc = bacc.Bacc("TRN2", target_bir_lowering=False, debug=False,
                   num_devices=NCORES)

    x_d = nc.dram_tensor("x", [S, L, E], F32, kind="ExternalInput")
    sl_d = nc.dram_tensor("sl", [S, 1], I32, kind="ExternalInput")
    wih_d = nc.dram_tensor("w_ih", [4, E], F32, kind="ExternalInput")
    whh_d = nc.dram_tensor("w_hh", [1, 4], F32, kind="ExternalInput")
    b2_d = nc.dram_tensor("b2", [1, 4], F32, kind="ExternalInput")
    out_d = nc.dram_tensor("out", [S, L, E], F32, kind="ExternalOutput")

    # [S, L, E] viewed as [(s k), t, e]
    x_v = x_d.ap().rearrange("s (k t) e -> (s k) t e", t=T)
    out_v = out_d.ap().rearrange("s (k t) e -> (s k) t e", t=T)

    with tile.TileContext(nc) as tc, ExitStack() as ctx:
        big = ctx.enter_context(tc.tile_pool(name="big", bufs=1))
        work = ctx.enter_context(tc.tile_pool(name="work", bufs=3))
        st = ctx.enter_context(tc.tile_pool(name="state", bufs=3))
        ppxt = ctx.enter_context(tc.tile_pool(name="ppxt", bufs=2, space="PSUM"))
        ppxg = ctx.enter_context(tc.tile_pool(name="ppxg", bufs=2, space="PSUM"))
        ppmisc = ctx.enter_context(tc.tile_pool(name="ppmisc", bufs=1, space="PSUM"))

        x_sb = big.tile([V, T, E], F32)
        xgall = big.tile([V, U, 4], F32)     # gate order i,f,g,o; slot u
        hsall = big.tile([V, U], F32)
        ident = big.tile([128, 128], F32)
        ones = big.tile([1, 128], F32)
        wih_sb = big.tile([4, E], F32)
        whh_sb = big.tile([1, 4], F32)
        b2_sb = big.tile([1, 4], F32)
        wT_sb = big.tile([128, 3, 4], F32)   # W_ih^T per E-chunk
        w4c = big.tile([V, 4], F32)          # W_hh broadcast to partitions
        sl_sb = big.tile([S, 1], I32)
        hseq = big.tile([S, L], F32)
        attn_v = big.tile([V, T], F32)

        # ---- constants / setup ----
        make_identity(nc, ident[:])
        nc.vector.memset(ones[:], 1.0)
        nc.sync.dma_start(wih_sb[:], wih_d.ap())
        nc.sync.dma_start(whh_sb[:], whh_d.ap())
        nc.sync.dma_start(b2_sb[:], b2_d.ap())
        nc.sync.dma_start(sl_sb[:], sl_d.ap())

        # broadcast W_hh row to 128 partitions: ones^T @ whh
        w4_ps = ppmisc.tile([128, 4], F32, tag="w4ps")
        nc.tensor.matmul(w4_ps[:], lhsT=ones[:], rhs=whh_sb[:],
                         start=True, stop=True)
        nc.vector.tensor_copy(out=w4c[:], in_=w4_ps[:])

        # W_ih^T chunks via PE transpose
        wT_ps = ppmisc.tile([128, 12], F32, tag="wTps")
        for j, (e0, cs) in enumerate(ECH):
            nc.tensor.matmul(wT_ps[0:cs, j * 4:(j + 1) * 4],
                             lhsT=wih_sb[:, e0:e0 + cs], rhs=ident[0:4, 0:4],
                             is_transpose=True, start=True, stop=True)
        for j, (e0, cs) in enumerate(ECH):
            nc.vector.tensor_copy(out=wT_sb[0:cs, j, :],
                                  in_=wT_ps[0:cs, j * 4:(j + 1) * 4])

        # ---- input DMA (16 chunks, warmup-feeding tiles first) ----
        dma_chunks = list(range(12, 16)) + list(range(0, 12))
        for d in dma_chunks:
            nc.sync.dma_start(x_sb[:, d * 8:(d + 1) * 8, :],
                              x_v[:, d * 8:(d + 1) * 8, :])

        # ---- xg for one t-slot tau; writes xgall[:, slot, :] ----
        def xg_tile(tau, slot):
            xT_ps = ppxt.tile([128, 384], F32, tag="xTps")
            for j, (e0, cs) in enumerate(ECH):
                nc.tensor.matmul(xT_ps[0:cs, j * 128:(j + 1) * 128],
                                 lhsT=x_sb[:, tau, e0:e0 + cs],
                                 rhs=ident[:], is_transpose=True,
                                 start=True, stop=True)
            xT_sb = work.tile([128, 384], F32, tag="xTsb")
            nc.vector.tensor_copy(out=xT_sb[:, 0:256], in_=xT_ps[:, 0:256])
            nc.scalar.copy(out=xT_sb[0:44, 256:384], in_=xT_ps[0:44, 256:384])
            xg_ps = ppxg.tile([128, 4], F32, tag="xgps")
            nc.tensor.matmul(xg_ps[:], lhsT=ones[:], rhs=b2_sb[:],
                             start=True, stop=False)
            for j, (e0, cs) in enumerate(ECH):
                nc.tensor.matmul(xg_ps[:],
                                 lhsT=xT_sb[0:cs, j * 128:(j + 1) * 128],
                                 rhs=wT_sb[0:cs, j, :],
                                 start=False, stop=(j == 2))
            nc.vector.tensor_copy(out=xgall[:, slot, :], in_=xg_ps[:])

        # tiles 96..127 fill slots 128..159 (they feed the warmup shift)
        for tau in range(96, 128):
            xg_tile(tau, W + tau)

        # warmup region: zero, then per-seq shift from the previous chunk tail
        nc.vector.memset(xgall[:, 0:W, :], 0.0)
        for s in range(S):
            p0 = s * NCH
            nc.sync.dma_start(xgall[p0 + 1:p0 + NCH, 0:W, :],
                              xgall[p0:p0 + NCH - 1, T:U, :])

        # ---- one scan step ----
        def scan_step(u):
            g4 = st.tile([V, 4], F32, tag="g4")
            nc.scalar.activation(g4[:, 0:1], hsall[:, u - 1:u] if u > 0 else zero1[:],
                                 Act.Sigmoid, bias=xgall[:, u, 0:1], scale=w4c[:, 0:1])
            nc.scalar.activation(g4[:, 1:2], hsall[:, u - 1:u] if u > 0 else zero1[:],
                                 Act.Sigmoid, bias=xgall[:, u, 1:2], scale=w4c[:, 1:2])
            nc.scalar.activation(g4[:, 2:3], hsall[:, u - 1:u] if u > 0 else zero1[:],
                                 Act.Tanh, bias=xgall[:, u, 2:3], scale=w4c[:, 2:3])
            nc.scalar.activation(g4[:, 3:4], hsall[:, u - 1:u] if u > 0 else zero1[:],
                                 Act.Sigmoid, bias=xgall[:, u, 3:4], scale=w4c[:, 3:4])
            ig = st.tile([V, 1], F32, tag="ig")
            nc.vector.tensor_scalar_mul(ig[:], g4[:, 2:3], g4[:, 0:1])
            c_new = st.tile([V, 1], F32, tag="c")
            nc.vector.scalar_tensor_tensor(c_new[:], in0=c_prev[0][:],
                                           scalar=g4[:, 1:2], in1=ig[:],
                                           op0=Alu.mult, op1=Alu.add)
            th = st.tile([V, 1], F32, tag="th")
            nc.scalar.activation(th[:], c_new[:], Act.Tanh)
            nc.vector.tensor_scalar_mul(hsall[:, u:u + 1], th[:], g4[:, 3:4])
            c_prev[0] = c_new

        zero1 = big.tile([V, 1], F32)
        nc.vector.memset(zero1[:], 0.0)
        c0 = st.tile([V, 1], F32, tag="c")
        nc.vector.memset(c0[:], 0.0)
        c_prev = [c0]

        for u in range(W):
            scan_step(u)
        for u in range(W, U):
            tau = u - W
            if tau < 96:
                xg_tile(tau, u)
            scan_step(u)

        # ---- softmax over L per sequence ----
        # gather hsall[:, W:U] -> hseq[s, k*T + t]
        nc.sync.dma_start(hseq[:].rearrange("s (k t) -> s k t", t=T),
                          hsall[:, W:U])
        slf = big.tile([S, 1], F32)
        nc.vector.tensor_copy(out=slf[:], in_=sl_sb[:])
        cmp = big.tile([S, 1], F32)
        nc.vector.tensor_scalar(cmp[:], slf[:], 0.0, None, Alu.is_gt)
        nc.vector.scalar_tensor_tensor(hseq[:, 0:1], in0=cmp[:], scalar=NEG,
                                       in1=hseq[:, 0:1],
                                       op0=Alu.mult, op1=Alu.add)
        negmax = big.tile([S, 1], F32)
        nc.vector.tensor_reduce(negmax[:], hseq[:], axis=mybir.AxisListType.X,
                                op=Alu.max, negate=True)
        sume = big.tile([S, 1], F32)
        nc.scalar.activation(hseq[:], hseq[:], Act.Exp, bias=negmax[:],
                             scale=1.0, accum_out=sume[:])
        rinv = big.tile([S, 1], F32)
        nc.vector.reciprocal(rinv[:], sume[:])
        nc.vector.tensor_scalar_mul(hseq[:], hseq[:], rinv[:])
        # scatter back to vseq layout
        nc.sync.dma_start(attn_v[:],
                          hseq[:].rearrange("s (k t) -> s k t", t=T))

        # ---- out = attn * x (in place on x_sb), then DMA out ----
        for d in range(16):
            for tau in range(d * 8, (d + 1) * 8):
                if tau % 2 == 0:
                    nc.vector.tensor_scalar_mul(x_sb[:, tau, :], x_sb[:, tau, :],
                                                attn_v[:, tau:tau + 1])
                else:
                    nc.scalar.activation(x_sb[:, tau, :], x_sb[:, tau, :],
                                         Act.Copy, scale=attn_v[:, tau:tau + 1])
            nc.sync.dma_start(out_v[:, d * 8:(d + 1) * 8, :],
                              x_sb[:, d * 8:(d + 1) * 8, :])

    nc.compile()
    return nc


def _get_nc():
    if "nc" not in _CACHE:
        _CACHE["nc"] = _build_nc()
    return _CACHE["nc"]


def make_in_maps(x, source_lengths, W_ih, W_hh, b_ih, b_hh):
    x = np.ascontiguousarray(np.asarray(x, dtype=np.float32))
    sl = np.asarray(source_lengths).astype(np.int32).reshape(B, 1)
    wih = np.ascontiguousarray(np.asarray(W_ih, dtype=np.float32))
    whh = np.ascontiguousarray(np.asarray(W_hh, dtype=np.float32).reshape(4, 1).T)
    b2 = (np.asarray(b_ih, dtype=np.float32)
          + np.asarray(b_hh, dtype=np.float32)).reshape(1, 4)
    in_maps = []
    for c in range(NCORES):
        in_maps.append({
            "x": np.ascontiguousarray(x[c * S:(c + 1) * S]),
            "sl": np.ascontiguousarray(sl[c * S:(c + 1) * S]),
            "w_ih": wih,
            "w_hh": whh,
            "b2": np.ascontiguousarray(b2),
        })
    return in_maps


def kernel(x, source_lengths, W_ih, W_hh, b_ih, b_hh):
    from concourse.bass_utils import run_bass_kernel_spmd

    nc = _get_nc()
    in_maps = make_in_maps(x, source_lengths, W_ih, W_hh, b_ih, b_hh)
    res = run_bass_kernel_spmd(nc, in_maps, core_ids=list(range(NCORES)))
    out = np.concatenate([res.results[c]["out"] for c in range(NCORES)], axis=0)
    return out


# revision 8
# speedup vs baseline: 21238.3120x; 21238.3120x over previous
# Trainium2 Bass kernel for nn_Attention_81028853007030
#
# Model: 1-unit LSTM over [B=64, L=2048, E=300] -> scores -> (buggy) mask ->
# softmax over L -> attn * x.
#
# Strategy:
#   - Pure data parallel over 8 cores (8 sequences per core).
#   - The LSTM recurrence (H=1) is chunked in time: each (seq, chunk-of-128)
#     pair becomes one of 128 SBUF partitions; every chunk re-runs a 32-step
#     warmup from zero state. Forget-gate decay makes the truncated state
#     influence < fp32 rounding (validated offline: h abs err ~7e-6, same as a
#     full fp32 scan).
#   - xg = x @ W_ih^T + b is computed on the TensorEngine (PE transpose of x
#     tiles + matmul), pipelined with the serial scan.
#   - Softmax per sequence with fused exp+sum on the scalar engine.
#   - Final attn*x multiply runs in-place on the resident x tile.

import numpy as np

B, L, E = 64, 2048, 300
NCORES = 8
S = B // NCORES          # sequences per core
T = 128                  # chunk length
NCH = L // T             # chunks per sequence
V = S * NCH              # virtual sequences per core = 128 partitions
W = 32                   # warmup steps
U = W + T                # scan steps
ECH = [(0, 128), (128, 128), (256, 44)]  # E-chunks for the matmul
NEG = -1.0e30

_CACHE = {}


def _build_nc(loop_n=0):
    from contextlib import ExitStack

    import concourse.bacc as bacc
    import concourse.mybir as mybir
    from concourse import tile
    from concourse.masks import make_identity

    F32 = mybir.dt.float32
    I32 = mybir.dt.int32
    Alu = mybir.AluOpType
    Act = mybir.ActivationFunctionType

    nc = bacc.Bacc("TRN2", target_bir_lowering=False, debug=False,
                   num_devices=NCORES)

    x_d = nc.dram_tensor("x", [S, L, E], F32, kind="ExternalInput")
    sl_d = nc.dram_tensor("sl", [S, 1], I32, kind="ExternalInput")
    wih_d = nc.dram_tensor("w_ih", [4, E], F32, kind="ExternalInput")
    whh_d = nc.dram_tensor("w_hh", [1, 4], F32, kind="ExternalInput")
    b2_d = nc.dram_tensor("b2", [1, 4], F32, kind="ExternalInput")
    out_d = nc.dram_tensor("out", [S, L, E], F32, kind="ExternalOutput")

    # [S, L, E] viewed as [(s k), t, e]
    x_v = x_d.ap().rearrange("s (k t) e -> (s k) t e", t=T)
    out_v = out_d.ap().rearrange("s (k t) e -> (s k) t e", t=T)

    with tile.TileContext(nc) as tc, ExitStack() as ctx:
        big = ctx.enter_context(tc.tile_pool(name="big", bufs=1))
        work = ctx.enter_context(tc.tile_pool(name="work", bufs=3))
        st = ctx.enter_context(tc.tile_pool(name="state", bufs=3))
        ppxt = ctx.enter_context(tc.tile_pool(name="ppxt", bufs=2, space="PSUM"))
        ppxg = ctx.enter_context(tc.tile_pool(name="ppxg", bufs=2, space="PSUM"))
        ppmisc = ctx.enter_context(tc.tile_pool(name="ppmisc", bufs=1, space="PSUM"))

        def emit_all():
            x_sb = big.tile([V, T, E], F32, tag="x_sb")
            xgall = big.tile([V, U, 4], F32, tag="xgall")  # gates i,f,g,o
            hsall = big.tile([V, U], F32, tag="hsall")
            ident = big.tile([128, 128], F32, tag="ident")
            ones = big.tile([1, 128], F32, tag="ones")
            wih_sb = big.tile([4, E], F32, tag="wih_sb")
            whh_sb = big.tile([1, 4], F32, tag="whh_sb")
            b2_sb = big.tile([1, 4], F32, tag="b2_sb")
            wT_sb = big.tile([128, 3, 4], F32, tag="wT_sb")
            w4c = big.tile([V, 4], F32, tag="w4c")
            sl_sb = big.tile([S, 1], I32, tag="sl_sb")
            hseq = big.tile([S, L], F32, tag="hseq")
            attn_v = big.tile([V, T], F32, tag="attn_v")
            zero1 = big.tile([V, 1], F32, tag="zero1")

            # ---- constants / setup ----
            make_identity(nc, ident[:])
            nc.vector.memset(ones[:], 1.0)
            nc.sync.dma_start(wih_sb[:], wih_d.ap())
            nc.sync.dma_start(whh_sb[:], whh_d.ap())
            nc.sync.dma_start(b2_sb[:], b2_d.ap())
            nc.sync.dma_start(sl_sb[:], sl_d.ap())

            # broadcast W_hh row to 128 partitions: ones^T @ whh
            w4_ps = ppmisc.tile([128, 4], F32, tag="w4ps")
            nc.tensor.matmul(w4_ps[:], lhsT=ones[:], rhs=whh_sb[:],
                             start=True, stop=True)
            nc.vector.tensor_copy(out=w4c[:], in_=w4_ps[:])

            # W_ih^T chunks via PE transpose
            wT_ps = ppmisc.tile([128, 12], F32, tag="wTps")
            for j, (e0, cs) in enumerate(ECH):
                nc.tensor.matmul(wT_ps[0:cs, j * 4:(j + 1) * 4],
                                 lhsT=wih_sb[:, e0:e0 + cs],
                                 rhs=ident[0:4, 0:4],
                                 is_transpose=True, start=True, stop=True)
            for j, (e0, cs) in enumerate(ECH):
                nc.vector.tensor_copy(out=wT_sb[0:cs, j, :],
                                      in_=wT_ps[0:cs, j * 4:(j + 1) * 4])

            # ---- input DMA (16 chunks, warmup-feeding tiles first) ----
            for d in list(range(12, 16)) + list(range(0, 12)):
                nc.sync.dma_start(x_sb[:, d * 8:(d + 1) * 8, :],
                                  x_v[:, d * 8:(d + 1) * 8, :])

            # ---- xg for one t-slot tau -> xgall[:, slot, :] ----
            def xg_tile(tau, slot):
                xT_ps = ppxt.tile([128, 384], F32, tag="xTps")
                for j, (e0, cs) in enumerate(ECH):
                    nc.tensor.matmul(xT_ps[0:cs, j * 128:(j + 1) * 128],
                                     lhsT=x_sb[:, tau, e0:e0 + cs],
                                     rhs=ident[:], is_transpose=True,
                                     start=True, stop=True)
                xT_sb = work.tile([128, 384], F32, tag="xTsb")
                nc.vector.tensor_copy(out=xT_sb[:, 0:256], in_=xT_ps[:, 0:256])
                nc.scalar.copy(out=xT_sb[0:44, 256:384],
                               in_=xT_ps[0:44, 256:384])
                xg_ps = ppxg.tile([128, 4], F32, tag="xgps")
                nc.tensor.matmul(xg_ps[:], lhsT=ones[:], rhs=b2_sb[:],
                                 start=True, stop=False)
                for j, (e0, cs) in enumerate(ECH):
                    nc.tensor.matmul(xg_ps[:],
                                     lhsT=xT_sb[0:cs, j * 128:(j + 1) * 128],
                                     rhs=wT_sb[0:cs, j, :],
                                     start=False, stop=(j == 2))
                nc.vector.tensor_copy(out=xgall[:, slot, :], in_=xg_ps[:])

            # tiles 96..127 fill slots 128..159 (feed the warmup shift)
            for tau in range(96, 128):
                xg_tile(tau, W + tau)

            # warmup: zero, then per-seq shift from the previous chunk's tail
            nc.vector.memset(xgall[:, 0:W, :], 0.0)
            for s in range(S):
                p0 = s * NCH
                nc.sync.dma_start(xgall[p0 + 1:p0 + NCH, 0:W, :],
                                  xgall[p0:p0 + NCH - 1, T:U, :])

            nc.vector.memset(zero1[:], 0.0)
            c0 = st.tile([V, 1], F32, tag="c")
            nc.vector.memset(c0[:], 0.0)
            c_prev = [c0]

            # ---- one scan step ----
            def scan_step(u):
                h_in = hsall[:, u - 1:u] if u > 0 else zero1[:]
                g4 = st.tile([V, 4], F32, tag="g4")
                nc.scalar.activation(g4[:, 1:2], h_in, Act.Sigmoid,
                                     bias=xgall[:, u, 1:2], scale=w4c[:, 1:2])
                nc.scalar.activation(g4[:, 0:1], h_in, Act.Sigmoid,
                                     bias=xgall[:, u, 0:1], scale=w4c[:, 0:1])
                nc.scalar.activation(g4[:, 2:3], h_in, Act.Tanh,
                                     bias=xgall[:, u, 2:3], scale=w4c[:, 2:3])
                nc.scalar.activation(g4[:, 3:4], h_in, Act.Sigmoid,
                                     bias=xgall[:, u, 3:4], scale=w4c[:, 3:4])
                ig = st.tile([V, 1], F32, tag="ig")
                nc.vector.tensor_scalar_mul(ig[:], g4[:, 2:3], g4[:, 0:1])
                c_new = st.tile([V, 1], F32, tag="c")
                nc.vector.scalar_tensor_tensor(c_new[:], in0=c_prev[0][:],
                                               scalar=g4[:, 1:2], in1=ig[:],
                                               op0=Alu.mult, op1=Alu.add)
                th = st.tile([V, 1], F32, tag="th")
                nc.scalar.activation(th[:], c_new[:], Act.Tanh)
                nc.vector.tensor_scalar_mul(hsall[:, u:u + 1], th[:],
                                            g4[:, 3:4])
                c_prev[0] = c_new

            for u in range(W):
                scan_step(u)
            for u in range(W, U):
                tau = u - W
                if tau < 96:
                    xg_tile(tau, u)
                scan_step(u)

            # ---- softmax over L per sequence ----
            nc.sync.dma_start(hseq[:].rearrange("s (k t) -> s k t", t=T),
                              hsall[:, W:U])
            slf = big.tile([S, 1], F32, tag="slf")
            nc.vector.tensor_copy(out=slf[:], in_=sl_sb[:])
            cmp = big.tile([S, 1], F32, tag="cmp")
            nc.vector.tensor_scalar(cmp[:], slf[:], 0.0, None, Alu.is_gt)
            nc.vector.scalar_tensor_tensor(hseq[:, 0:1], in0=cmp[:],
                                           scalar=NEG, in1=hseq[:, 0:1],
                                           op0=Alu.mult, op1=Alu.add)
            negmax = big.tile([S, 1], F32, tag="negmax")
            nc.vector.tensor_reduce(negmax[:], hseq[:],
                                    axis=mybir.AxisListType.X,
                                    op=Alu.max, negate=True)
            sume = big.tile([S, 1], F32, tag="sume")
            nc.scalar.activation(hseq[:], hseq[:], Act.Exp, bias=negmax[:],
                                 scale=1.0, accum_out=sume[:])
            rinv = big.tile([S, 1], F32, tag="rinv")
            nc.vector.reciprocal(rinv[:], sume[:])
            nc.vector.tensor_scalar_mul(hseq[:], hseq[:], rinv[:])
            nc.sync.dma_start(attn_v[:],
                              hseq[:].rearrange("s (k t) -> s k t", t=T))

            # ---- out = attn * x (in place), then DMA out ----
            for d in range(16):
                for tau in range(d * 8, (d + 1) * 8):
                    if tau % 2 == 0:
                        nc.vector.tensor_scalar_mul(x_sb[:, tau, :],
                                                    x_sb[:, tau, :],
                                                    attn_v[:, tau:tau + 1])
                    else:
                        nc.scalar.activation(x_sb[:, tau, :], x_sb[:, tau, :],
                                             Act.Copy,
                                             scale=attn_v[:, tau:tau + 1])
                nc.sync.dma_start(out_v[:, d * 8:(d + 1) * 8, :],
                                  x_sb[:, d * 8:(d + 1) * 8, :])

        if loop_n:
            with tc.For_i(0, loop_n, 1):
                emit_all()
        else:
            emit_all()

    nc.compile()
    return nc


def _get_nc(loop_n=0):
    key = ("nc", loop_n)
    if key not in _CACHE:
        _CACHE[key] = _build_nc(loop_n)
    return _CACHE[key]


def make_in_maps(x, source_lengths, W_ih, W_hh, b_ih, b_hh):
    x = np.ascontiguousarray(np.asarray(x, dtype=np.float32))
    sl = np.asarray(source_lengths).astype(np.int32).reshape(B, 1)
    wih = np.ascontiguousarray(np.asarray(W_ih, dtype=np.float32))
    whh = np.ascontiguousarray(np.asarray(W_hh, dtype=np.float32).reshape(4, 1).T)
    b2 = (np.asarray(b_ih, dtype=np.float32)
          + np.asarray(b_hh, dtype=np.float32)).reshape(1, 4)
    in_maps = []
    for c in range(NCORES):
        in_maps.append({
            "x": np.ascontiguousarray(x[c * S:(c + 1) * S]),
            "sl": np.ascontiguousarray(sl[c * S:(c + 1) * S]),
            "w_ih": wih,
            "w_hh": whh,
            "b2": np.ascontiguousarray(b2),
        })
    return in_maps


def kernel(x, source_lengths, W_ih, W_hh, b_ih, b_hh):
    from concourse.bass_utils import run_bass_kernel_spmd

    nc = _get_nc()
    in_maps = make_in_maps(x, source_lengths, W_ih, W_hh, b_ih, b_hh)
    res = run_bass_kernel_spmd(nc, in_maps, core_ids=list(range(NCORES)))
    out = np.concatenate([res.results[c]["out"] for c in range(NCORES)], axis=0)
    return out


# revision 9
# speedup vs baseline: 135879.3932x; 6.3978x over previous
# Trainium2 Bass kernel for nn_Attention_81028853007030
#
# Model: 1-unit LSTM over [B=64, L=2048, E=300] -> scores -> (buggy) mask ->
# softmax over L -> attn * x.
#
# Strategy:
#   - Pure data parallel over 8 cores (8 sequences per core).
#   - The LSTM recurrence (H=1) is chunked in time: each (seq, chunk) pair is
#     one of 128 SBUF partitions, and the 128 t-slots split into WAVES
#     independent scan chains (chunk length TCH) that interleave on the
#     engines, hiding per-step latency. Every chunk re-runs a WM=16-step
#     warmup from zero state; forget-gate decay makes the truncation error
#     < fp32 rounding (validated offline vs fp64).
#   - xg = x @ W_ih^T + b on the TensorEngine (PE transpose + matmul),
#     pipelined with the scan. Gate order is permuted to i,f,o,g so one
#     sigmoid covers i,f,o and one tanh covers g.
#   - Softmax per sequence with fused exp+sum; final attn*x in place.

import numpy as np

B, L, E = 64, 2048, 300
NCORES = 8
S = B // NCORES          # sequences per core
WAVES = 4                # independent scan chains
TCH = 128 // WAVES       # chunk length per wave
WM = 16                  # warmup steps
UW = WM + TCH            # steps per wave
NCH = L // TCH           # chunks per sequence
V = 128                  # partitions = S * (16 chunks-of-128)
ECH = [(0, 128), (128, 128), (256, 44)]  # E-chunks for the matmul
NEG = -1.0e30

_CACHE = {}


def _build_nc(loop_n=0):
    from contextlib import ExitStack

    import concourse.bacc as bacc
    import concourse.mybir as mybir
    from concourse import tile
    from concourse.masks import make_identity

    F32 = mybir.dt.float32
    I32 = mybir.dt.int32
    Alu = mybir.AluOpType
    Act = mybir.ActivationFunctionType

    nc = bacc.Bacc("TRN2", target_bir_lowering=False, debug=False,
                   num_devices=NCORES)

    x_d = nc.dram_tensor("x", [S, L, E], F32, kind="ExternalInput")
    sl_d = nc.dram_tensor("sl", [S, 1], I32, kind="ExternalInput")
    wih_d = nc.dram_tensor("w_ih", [4, E], F32, kind="ExternalInput")
    whh_d = nc.dram_tensor("w_hh", [1, 4], F32, kind="ExternalInput")
    b2_d = nc.dram_tensor("b2", [1, 4], F32, kind="ExternalInput")
    out_d = nc.dram_tensor("out", [S, L, E], F32, kind="ExternalOutput")

    # [S, L, E] viewed as [(s k128), t, e]; partition p = s*16 + j covers
    # L-rows j*128 .. j*128+127 of sequence s; t-slot tau = w*TCH + t holds
    # wave w's chunk element t.
    x_v = x_d.ap().rearrange("s (k t) e -> (s k) t e", t=128)
    out_v = out_d.ap().rearrange("s (k t) e -> (s k) t e", t=128)

    with tile.TileContext(nc) as tc, ExitStack() as ctx:
        big = ctx.enter_context(tc.tile_pool(name="big", bufs=1))
        work = ctx.enter_context(tc.tile_pool(name="work", bufs=3))
        st = ctx.enter_context(tc.tile_pool(name="state", bufs=3))
        ppxt = ctx.enter_context(tc.tile_pool(name="ppxt", bufs=2, space="PSUM"))
        ppxg = ctx.enter_context(tc.tile_pool(name="ppxg", bufs=2, space="PSUM"))
        ppmisc = ctx.enter_context(tc.tile_pool(name="ppmisc", bufs=1, space="PSUM"))

        def emit_all():
            x_sb = big.tile([V, 128, E], F32, tag="x_sb")
            xga = big.tile([V, WAVES, UW, 4], F32, tag="xga")  # i,f,o,g
            hwarm = big.tile([V, WAVES, WM], F32, tag="hwarm")
            hs128 = big.tile([V, 128], F32, tag="hs128")
            ident = big.tile([128, 128], F32, tag="ident")
            ones = big.tile([1, 128], F32, tag="ones")
            wih_sb = big.tile([4, E], F32, tag="wih_sb")
            whh_sb = big.tile([1, 4], F32, tag="whh_sb")
            b2_sb = big.tile([1, 4], F32, tag="b2_sb")
            wT_sb = big.tile([128, 3, 4], F32, tag="wT_sb")
            w4c = big.tile([V, 4], F32, tag="w4c")
            sl_sb = big.tile([S, 1], I32, tag="sl_sb")
            hseq = big.tile([S, L], F32, tag="hseq")
            attn_v = big.tile([V, 128], F32, tag="attn_v")
            zero1 = big.tile([V, 1], F32, tag="zero1")

            # ---- constants / setup ----
            make_identity(nc, ident[:])
            nc.vector.memset(ones[:], 1.0)
            nc.sync.dma_start(wih_sb[:], wih_d.ap())
            nc.sync.dma_start(whh_sb[:], whh_d.ap())
            nc.sync.dma_start(b2_sb[:], b2_d.ap())
            nc.sync.dma_start(sl_sb[:], sl_d.ap())

            w4_ps = ppmisc.tile([128, 4], F32, tag="w4ps")
            nc.tensor.matmul(w4_ps[:], lhsT=ones[:], rhs=whh_sb[:],
                             start=True, stop=True)
            nc.vector.tensor_copy(out=w4c[:], in_=w4_ps[:])

            wT_ps = ppmisc.tile([128, 12], F32, tag="wTps")
            for j, (e0, cs) in enumerate(ECH):
                nc.tensor.matmul(wT_ps[0:cs, j * 4:(j + 1) * 4],
                                 lhsT=wih_sb[:, e0:e0 + cs],
                                 rhs=ident[0:4, 0:4],
                                 is_transpose=True, start=True, stop=True)
            for j, (e0, cs) in enumerate(ECH):
                nc.vector.tensor_copy(out=wT_sb[0:cs, j, :],
                                      in_=wT_ps[0:cs, j * 4:(j + 1) * 4])

            # ---- input DMA: tail-feeding chunks first ----
            tails = [w * TCH + t for t in range(TCH - WM, TCH)
                     for w in range(WAVES)]
            heads = [w * TCH + t for t in range(0, TCH - WM)
                     for w in range(WAVES)]
            d_order = []
            for tau in tails + heads:
                d = tau // 8
                if d not in d_order:
                    d_order.append(d)
            for d in d_order:
                nc.sync.dma_start(x_sb[:, d * 8:(d + 1) * 8, :],
                                  x_v[:, d * 8:(d + 1) * 8, :])

            # ---- xg for t-slot tau -> xga[:, tau//TCH, WM + tau%TCH, :] ----
            def xg_tile(tau):
                w, t = tau // TCH, tau % TCH
                xT_ps = ppxt.tile([128, 384], F32, tag="xTps")
                for j, (e0, cs) in enumerate(ECH):
                    nc.tensor.matmul(xT_ps[0:cs, j * 128:(j + 1) * 128],
                                     lhsT=x_sb[:, tau, e0:e0 + cs],
                                     rhs=ident[:], is_transpose=True,
                                     start=True, stop=True)
                xT_sb = work.tile([128, 384], F32, tag="xTsb")
                nc.vector.tensor_copy(out=xT_sb[:, 0:256], in_=xT_ps[:, 0:256])
                nc.scalar.copy(out=xT_sb[0:44, 256:384],
                               in_=xT_ps[0:44, 256:384])
                xg_ps = ppxg.tile([128, 4], F32, tag="xgps")
                nc.tensor.matmul(xg_ps[:], lhsT=ones[:], rhs=b2_sb[:],
                                 start=True, stop=False)
                for j, (e0, cs) in enumerate(ECH):
                    nc.tensor.matmul(xg_ps[:],
                                     lhsT=xT_sb[0:cs, j * 128:(j + 1) * 128],
                                     rhs=wT_sb[0:cs, j, :],
                                     start=False, stop=(j == 2))
                nc.vector.tensor_copy(out=xga[:, w, WM + t, :], in_=xg_ps[:])

            for tau in tails:
                xg_tile(tau)

            # ---- warmup slots ----
            # wave 0 chunk j warms up from wave WAVES-1 chunk j-1 (partition
            # p-1); chunk 0 of each sequence keeps zeros.
            nc.vector.memset(xga[:, 0, 0:WM, :], 0.0)
            for s in range(S):
                p0 = s * 16
                nc.sync.dma_start(xga[p0 + 1:p0 + 16, 0, 0:WM, :],
                                  xga[p0:p0 + 15, WAVES - 1, TCH:UW, :])
            # wave w >= 1 warms up from wave w-1 same partition
            for w in range(1, WAVES):
                nc.vector.tensor_copy(out=xga[:, w, 0:WM, :],
                                      in_=xga[:, w - 1, TCH:UW, :])

            nc.vector.memset(zero1[:], 0.0)
            c_prev = []
            for w in range(WAVES):
                c0 = st.tile([V, 1], F32, tag=f"c{w}")
                nc.vector.memset(c0[:], 0.0)
                c_prev.append(c0)

            # ---- one scan step of wave w ----
            def scan_step(w, u):
                if u == 0:
                    h_in = zero1[:]
                elif u - 1 < WM:
                    h_in = hwarm[:, w, u - 1:u]
                else:
                    tau = w * TCH + (u - 1 - WM)
                    h_in = hs128[:, tau:tau + 1]
                p4 = st.tile([V, 4], F32, tag=f"p4{w}")
                nc.vector.scalar_tensor_tensor(p4[:], in0=w4c[:],
                                               scalar=h_in,
                                               in1=xga[:, w, u, :],
                                               op0=Alu.mult, op1=Alu.add)
                g4 = st.tile([V, 4], F32, tag=f"g4{w}")
                nc.scalar.activation(g4[:, 0:3], p4[:, 0:3], Act.Sigmoid)
                nc.scalar.activation(g4[:, 3:4], p4[:, 3:4], Act.Tanh)
                ig = st.tile([V, 1], F32, tag=f"ig{w}")
                nc.vector.tensor_scalar_mul(ig[:], g4[:, 3:4], g4[:, 0:1])
                c_new = st.tile([V, 1], F32, tag=f"c{w}")
                nc.vector.scalar_tensor_tensor(c_new[:], in0=c_prev[w][:],
                                               scalar=g4[:, 1:2], in1=ig[:],
                                               op0=Alu.mult, op1=Alu.add)
                th = st.tile([V, 1], F32, tag=f"th{w}")
                nc.scalar.activation(th[:], c_new[:], Act.Tanh)
                if u < WM:
                    h_out = hwarm[:, w, u:u + 1]
                else:
                    h_out = hs128[:, w * TCH + (u - WM):w * TCH + (u - WM) + 1]
                nc.vector.tensor_scalar_mul(h_out, th[:], g4[:, 2:3])
                c_prev[w] = c_new

            for u in range(WM):
                for w in range(WAVES):
                    scan_step(w, u)
            for u in range(WM, UW):
                if u < TCH:
                    for w in range(WAVES):
                        xg_tile(w * TCH + (u - WM))
                for w in range(WAVES):
                    scan_step(w, u)

            # ---- softmax over L per sequence ----
            nc.sync.dma_start(hseq[:].rearrange("s (k t) -> s k t", t=128),
                              hs128[:])
            slf = big.tile([S, 1], F32, tag="slf")
            nc.vector.tensor_copy(out=slf[:], in_=sl_sb[:])
            cmp = big.tile([S, 1], F32, tag="cmp")
            nc.vector.tensor_scalar(cmp[:], slf[:], 0.0, None, Alu.is_gt)
            nc.vector.scalar_tensor_tensor(hseq[:, 0:1], in0=cmp[:],
                                           scalar=NEG, in1=hseq[:, 0:1],
                                           op0=Alu.mult, op1=Alu.add)
            negmax = big.tile([S, 1], F32, tag="negmax")
            nc.vector.tensor_reduce(negmax[:], hseq[:],
                                    axis=mybir.AxisListType.X,
                                    op=Alu.max, negate=True)
            sume = big.tile([S, 1], F32, tag="sume")
            nc.scalar.activation(hseq[:], hseq[:], Act.Exp, bias=negmax[:],
                                 scale=1.0, accum_out=sume[:])
            rinv = big.tile([S, 1], F32, tag="rinv")
            nc.vector.reciprocal(rinv[:], sume[:])
            nc.vector.tensor_scalar_mul(hseq[:], hseq[:], rinv[:])
            nc.sync.dma_start(attn_v[:],
                              hseq[:].rearrange("s (k t) -> s k t", t=128))

            # ---- out = attn * x (in place), then DMA out ----
            for d in range(16):
                for tau in range(d * 8, (d + 1) * 8):
                    if tau % 2 == 0:
                        nc.vector.tensor_scalar_mul(x_sb[:, tau, :],
                                                    x_sb[:, tau, :],
                                                    attn_v[:, tau:tau + 1])
                    else:
                        nc.scalar.activation(x_sb[:, tau, :], x_sb[:, tau, :],
                                             Act.Copy,
                                             scale=attn_v[:, tau:tau + 1])
                nc.sync.dma_start(out_v[:, d * 8:(d + 1) * 8, :],
                                  x_sb[:, d * 8:(d + 1) * 8, :])

        if loop_n:
            with tc.For_i(0, loop_n, 1):
                emit_all()
        else:
            emit_all()

    nc.compile()
    return nc


def _get_nc(loop_n=0):
    key = ("nc", loop_n)
    if key not in _CACHE:
        _CACHE[key] = _build_nc(loop_n)
    return _CACHE[key]


# gate order i,f,g,o -> i,f,o,g
_PERM = [0, 1, 3, 2]


def make_in_maps(x, source_lengths, W_ih, W_hh, b_ih, b_hh):
    x = np.ascontiguousarray(np.asarray(x, dtype=np.float32))
    sl = np.asarray(source_lengths).astype(np.int32).reshape(B, 1)
    wih = np.ascontiguousarray(np.asarray(W_ih, dtype=np.float32)[_PERM])
    whh = np.ascontiguousarray(
        np.asarray(W_hh, dtype=np.float32).reshape(4)[_PERM].reshape(1, 4))
    b2 = (np.asarray(b_ih, dtype=np.float32)
          + np.asarray(b_hh, dtype=np.float32))[_PERM].reshape(1, 4)
    in_maps = []
    for c in range(NCORES):
        in_maps.append({
            "x": np.ascontiguousarray(x[c * S:(c + 1) * S]),
            "sl": np.ascontiguousarray(sl[c * S:(c + 1) * S]),
            "w_ih": wih,
            "w_hh": whh,
            "b2": np.ascontiguousarray(b2),
        })
    return in_maps


def kernel(x, source_lengths, W_ih, W_hh, b_ih, b_hh):
    from concourse.bass_utils import run_bass_kernel_spmd

    nc = _get_nc()
    in_maps = make_in_maps(x, source_lengths, W_ih, W_hh, b_ih, b_hh)
    res = run_bass_kernel_spmd(nc, in_maps, core_ids=list(range(NCORES)))
    out = np.concatenate([res.results[c]["out"] for c in range(NCORES)], axis=0)
    return out
